# revision 22
# baseline (speedup 1.0000x reference)
"""Trainium2 Bass kernel for an expert-choice MoR block (B=4, S=4096, D=2048, F=8192).

Sharding: 8 cores = 4 batch rows x 2 token-halves. Each core redundantly
computes the router + top-k selection for its batch row on-device (pairwise
rank counting on DVE+ACT, gpsimd sparse_gather compaction), gathers its half
of the selected tokens by rank-range (indirect DMA), runs the SiLU FFN in
bf16 on the PE, applies the router weights, and emits its 1024 processed
rows + the index lists + the row's z-loss term. The host only splits inputs
and scatters the per-core rows back into a copy of hidden_states.
"""

import numpy as np
import ml_dtypes

B, S, D, F = 4, 4096, 2048, 8192
K = S // 2          # 2048 selected tokens per batch row
KC = K // 2         # 1024 tokens per core
P = 128
S_TILES = S // P    # 32
TOK_TILES = KC // P # 8
D_CHUNKS = D // P   # 16
N_SUPER = 16        # F split into supers of 512
FS = F // N_SUPER   # 512
FC_PER_SUPER = FS // P  # 4

_COMPILED = {}


def build_program():
    import concourse.bass as bass
    import concourse.tile as tile
    import concourse.bass_isa as bass_isa
    from concourse import mybir, bacc

    fp32 = mybir.dt.float32
    bf16 = mybir.dt.bfloat16
    i16 = mybir.dt.int16
    i32 = mybir.dt.int32
    u32 = mybir.dt.uint32
    Alu = mybir.AluOpType
    Act = mybir.ActivationFunctionType
    Ax = mybir.AxisListType

    nc = bacc.Bacc("TRN2", target_bir_lowering=False, debug=False)

    hid = nc.dram_tensor("hid", [S, D], fp32, kind="ExternalInput").ap()
    wr = nc.dram_tensor("wr", [D], fp32, kind="ExternalInput").ap()
    w1b = nc.dram_tensor("w1b", [D, F], bf16, kind="ExternalInput").ap()
    w2b = nc.dram_tensor("w2b", [F, D], bf16, kind="ExternalInput").ap()
    cons = nc.dram_tensor("cons", [P, 4], fp32, kind="ExternalInput").ap()

    rows_o = nc.dram_tensor("rows", [KC, D], fp32, kind="ExternalOutput").ap()
    myidx_o = nc.dram_tensor("myidx", [KC], i32, kind="ExternalOutput").ap()
    selall_o = nc.dram_tensor("selall", [K], i32, kind="ExternalOutput").ap()
    zsq_o = nc.dram_tensor("zsq", [1], fp32, kind="ExternalOutput").ap()

    with tile.TileContext(nc) as tc:
        with tc.tile_pool(name="persist", bufs=1) as pp:
            ident = pp.tile([P, P], fp32, tag="ident")
            identb = pp.tile([P, P], bf16, tag="identb")
            ones_pp = pp.tile([P, P], fp32, tag="ones_pp")
            nc.gpsimd.memset(ones_pp[:], 1.0)
            # identity: keep 1.0 where (partition - col) == 0
            nc.gpsimd.affine_select(
                ident[:], ones_pp[:], pattern=[[-1, P]], compare_op=Alu.is_equal,
                fill=0.0, base=0, channel_multiplier=1,
            )
            nc.vector.tensor_copy(identb[:], ident[:])

            consb = pp.tile([P, 4], fp32, tag="consb")
            nc.sync.dma_start(consb[:], cons[:])

            idx_rep = pp.tile([P, KC // 16], i16, tag="idx_rep")      # [128, 64]
            w_col = pp.tile([P, TOK_TILES], fp32, tag="w_col")        # [128, 8]
            logits_sb = pp.tile([P, S_TILES], fp32, tag="logits_sb")  # [128, 32]

            # ---------------- Phase A: router, ranks, selection ----------------
            with (
                tc.tile_pool(name="pa_big", bufs=3) as pa_big,
                tc.tile_pool(name="pa_bc", bufs=1) as pa_bc,
                tc.tile_pool(name="pa_junk", bufs=2) as pa_junk,
                tc.tile_pool(name="pa_junk2", bufs=2) as pa_junk2,
                tc.tile_pool(name="pa_small", bufs=1) as pa_small,
                tc.tile_pool(name="pa_tiny", bufs=2) as pa_tiny,
                tc.tile_pool(name="pa_psum", bufs=1, space="PSUM") as pa_psum,
            ):
                wr_row = pa_small.tile([1, D], fp32, tag="wr_row")
                nc.sync.dma_start(wr_row[:], wr[:])
                wrb = pa_bc.tile([P, D], fp32, tag="wrb")
                nc.gpsimd.partition_broadcast(wrb[:], wr_row[:], channels=P)

                # router logits: two-stage segmented reduction (bounds the
                # sequential-sum rounding walk). Two row-tiles per op group
                # to amortize DVE instruction overhead. Processed in two
                # 2048-token halves so the pairwise ranking of the first half
                # overlaps the router work of the second.
                HT = S_TILES // 2  # 16 tiles per half
                SH = S // 2        # 2048 tokens per half
                wrb2 = wrb[:].rearrange("p (t d) -> p t d", t=1).broadcast_to([P, 2, D])
                neg_logits = pa_small.tile([P, S_TILES], fp32, tag="neg_logits")
                hid_v = hid.rearrange("(i q p) d -> p i q d", p=P, q=2)
                lbc_halves = []

                def router_half(h):
                    for i in range(h * HT, (h + 1) * HT, 2):
                        ht = pa_big.tile([P, 2, D], fp32, tag="ht")
                        nc.sync.dma_start(ht[:], hid_v[:, i // 2])
                        prod = pa_big.tile([P, 2, D], fp32, tag="prod")
                        # 1 of 4 product groups on gpsimd, rest on DVE
                        if (i // 2) % 4 == 3:
                            nc.gpsimd.tensor_mul(prod[:], ht[:], wrb2)
                        else:
                            nc.vector.tensor_mul(prod[:], ht[:], wrb2)
                        # segmented accumulate on ACT (512-wide chains)
                        part8 = pa_tiny.tile([P, 2, 4], fp32, tag="part8")
                        for t in range(2):
                            for sg in range(4):
                                jr = pa_tiny.tile([P, 512], bf16, tag="jr")
                                nc.scalar.activation(
                                    jr[:], prod[:, t, sg * 512:(sg + 1) * 512],
                                    Act.Copy, bias=0.0, scale=1.0,
                                    accum_out=part8[:, t, sg:sg + 1],
                                )
                        nc.vector.tensor_reduce(
                            logits_sb[:, i:i + 2], part8[:], axis=Ax.X,
                            op=Alu.add,
                        )
                    nc.vector.tensor_scalar_mul(
                        neg_logits[:, h * HT:(h + 1) * HT],
                        logits_sb[:, h * HT:(h + 1) * HT], -1.0)
                    # flatten this half's logits to a row and broadcast
                    lrow = pa_small.tile([1, SH], fp32, tag=f"lrow{h}")
                    for j in range(4):
                        ps_row = pa_psum.tile([1, 512], fp32, tag="ps_row")
                        for q in range(4):
                            i = h * HT + 4 * j + q
                            nc.tensor.matmul(
                                ps_row[:, q * P:(q + 1) * P],
                                logits_sb[:, i:i + 1], ident[:],
                                start=True, stop=True,
                            )
                        nc.scalar.copy(lrow[:, j * 512:(j + 1) * 512], ps_row[:])
                    lbc = pa_bc.tile([P, SH], fp32, tag=f"lbc{h}")
                    nc.gpsimd.partition_broadcast(lbc[:], lrow[:], channels=P)
                    lbc_halves.append(lbc)

                # per-(i-tile, j-half) partial counts; ACT blocks use the
                # Sign trick, DVE blocks count is_gt directly.
                cnt = [pa_small.tile([P, HT], fp32, tag=f"cnt{jh}{half}",
                                     name=f"cnt{jh}{half}")
                       for jh in range(2) for half in range(2)]  # [jh*2+ih]

                def rank_block_act(itile, jh, out_col):
                    ja = pa_junk2.tile([P, SH], bf16, tag="ja")
                    nc.scalar.activation(
                        ja[:], lbc_halves[jh], Act.Sign,
                        bias=neg_logits[:, itile:itile + 1], scale=1.0,
                        accum_out=out_col,
                    )

                def rank_block_cnt(eng, itile, jh, out_col):
                    jd = pa_junk.tile([P, SH], bf16, tag="jd")
                    eng.tensor_scalar(
                        jd[:], lbc_halves[jh], logits_sb[:, itile:itile + 1],
                        None, op0=Alu.is_gt, op1=Alu.add, accum_out=out_col,
                    )

                router_half(0)
                router_half(1)
                # block A (ACT sign, z=1): i in H0 vs j-half 0
                for i in range(HT):
                    rank_block_act(i, 0, cnt[0][:, i:i + 1])
                # block C (DVE is_gt): i in H1 vs j-half 0
                for i in range(HT):
                    rank_block_cnt(nc.vector, HT + i, 0, cnt[1][:, i:i + 1])
                # block B: i in H0 vs j-half 1 — DVE head, ACT tail (z=0)
                B_DVE = 12
                for i in range(B_DVE):
                    rank_block_cnt(nc.vector, i, 1, cnt[2][:, i:i + 1])
                for i in range(B_DVE, HT):
                    rank_block_act(i, 1, cnt[2][:, i:i + 1])
                # block D (ACT sign, z=1): i in H1 vs j-half 1
                for i in range(HT):
                    rank_block_act(HT + i, 1, cnt[3][:, i:i + 1])

                # sign-sum -> gt-count conversion: cnt = (sgn + SH - z) / 2
                # z = 1 when the i-tile's half equals the j-half (self term)
                nc.vector.tensor_scalar(
                    cnt[0][:], cnt[0][:], float(SH - 1), 0.5,
                    op0=Alu.add, op1=Alu.mult)
                nc.vector.tensor_scalar(
                    cnt[2][:, B_DVE:HT], cnt[2][:, B_DVE:HT], float(SH), 0.5,
                    op0=Alu.add, op1=Alu.mult)
                nc.vector.tensor_scalar(
                    cnt[3][:], cnt[3][:], float(SH - 1), 0.5,
                    op0=Alu.add, op1=Alu.mult)

                ranks = pa_small.tile([P, S_TILES], fp32, tag="ranks")
                nc.vector.tensor_add(ranks[:, 0:HT], cnt[0][:], cnt[2][:])
                nc.vector.tensor_add(ranks[:, HT:S_TILES], cnt[1][:], cnt[3][:])

                # probs = sigmoid(logits) * 0.1
                probs = pa_small.tile([P, S_TILES], fp32, tag="probs")
                nc.scalar.activation(probs[:], logits_sb[:], Act.Sigmoid)
                nc.vector.tensor_scalar_mul(probs[:], probs[:], 0.1)

                # --- critical path to the gather: my_mask -> my_idx -> idx_rep
                W16 = S // 16  # 256
                iota1_i = pa_small.tile([16, W16], i32, tag="iota1_i")
                nc.gpsimd.iota(
                    iota1_i[:], pattern=[[16, W16]], base=1, channel_multiplier=1)
                iota1 = pa_small.tile([16, W16], fp32, tag="iota1")
                nc.vector.tensor_copy(iota1[:], iota1_i[:])

                ranks_w = pa_small.tile([16, S // 16], fp32, tag="ranks_w")
                rw_v = ranks_w[:].rearrange("p (f g) -> p f g", g=8)
                for g in range(8):
                    nc.sync.dma_start(
                        rw_v[:, :, g], ranks[16 * g:16 * (g + 1), :])

                ge = pa_small.tile([16, W16], fp32, tag="ge")
                nc.vector.tensor_scalar(
                    ge[:], ranks_w[:], consb[0:16, 0:1], None, op0=Alu.is_ge)
                lt = pa_small.tile([16, W16], fp32, tag="lt")
                nc.vector.tensor_scalar(
                    lt[:], ranks_w[:], consb[0:16, 1:2], None, op0=Alu.is_lt)
                my_mask = pa_small.tile([16, W16], fp32, tag="my_mask")
                nc.vector.tensor_mul(my_mask[:], ge[:], lt[:])

                mmy = pa_small.tile([16, W16], fp32, tag="mmy")
                nc.vector.tensor_mul(mmy[:], iota1[:], my_mask[:])
                nc.vector.tensor_scalar_add(mmy[:], mmy[:], -1.0)
                my_idx_f = pa_small.tile([16, KC // 16], fp32, tag="my_idx_f")
                nf2 = pa_small.tile([1, 1], u32, tag="nf2")
                nc.gpsimd.sparse_gather(my_idx_f[:], mmy[:], num_found=nf2[:])
                my_idx_s = pa_small.tile([16, KC // 16], i16, tag="my_idx_s")
                nc.vector.tensor_copy(my_idx_s[:], my_idx_f[:])
                for g in range(8):
                    nc.sync.dma_start(
                        idx_rep[16 * g:16 * (g + 1), :], my_idx_s[:])

                # --- off the critical path: sel/weights outputs
                probs_w = pa_small.tile([16, S // 16], fp32, tag="probs_w")
                pw_v = probs_w[:].rearrange("p (f g) -> p f g", g=8)
                for g in range(8):
                    nc.sync.dma_start(
                        pw_v[:, :, g], probs[16 * g:16 * (g + 1), :])
                sel_mask = pa_small.tile([16, W16], fp32, tag="sel_mask")
                nc.vector.tensor_scalar(
                    sel_mask[:], ranks_w[:], float(K), None, op0=Alu.is_lt)
                msel = pa_small.tile([16, W16], fp32, tag="msel")
                nc.vector.tensor_mul(msel[:], iota1[:], sel_mask[:])
                nc.vector.tensor_scalar_add(msel[:], msel[:], -1.0)
                # mprob = mask ? probs : -1  == (probs + 1) * mask - 1
                # (probs+1 rounds at ~1.2e-7 abs; negligible for the gating)
                mprob = pa_small.tile([16, W16], fp32, tag="mprob")
                nc.vector.tensor_scalar_add(mprob[:], probs_w[:], 1.0)
                nc.vector.tensor_mul(mprob[:], mprob[:], my_mask[:])
                nc.vector.tensor_scalar_add(mprob[:], mprob[:], -1.0)

                sel_all_f = pa_small.tile([16, K // 16], fp32, tag="sel_all_f")
                nf1 = pa_small.tile([1, 1], u32, tag="nf1")
                nc.gpsimd.sparse_gather(sel_all_f[:], msel[:], num_found=nf1[:])
                my_w_f = pa_small.tile([16, KC // 16], fp32, tag="my_w_f")
                nf3 = pa_small.tile([1, 1], u32, tag="nf3")
                nc.gpsimd.sparse_gather(my_w_f[:], mprob[:], num_found=nf3[:])

                sel_all_i = pa_small.tile([16, K // 16], i32, tag="sel_all_i")
                nc.vector.tensor_copy(sel_all_i[:], sel_all_f[:])
                nc.sync.dma_start(
                    selall_o.rearrange("(f p) -> p f", p=16), sel_all_i[:])
                my_idx_i = pa_small.tile([16, KC // 16], i32, tag="my_idx_i")
                nc.vector.tensor_copy(my_idx_i[:], my_idx_f[:])
                nc.sync.dma_start(
                    myidx_o.rearrange("(f p) -> p f", p=16), my_idx_i[:])

                # per-token weights wrapped-16 -> wrapped-128 column layout
                mw_v = my_w_f[:].rearrange("p (f g) -> p f g", g=8)
                for g in range(8):
                    nc.sync.dma_start(
                        w_col[16 * g:16 * (g + 1), :], mw_v[:, :, g])

                # z-loss = logsumexp(logits)^2
                m8 = pa_tiny.tile([P, 1], fp32, tag="m8")
                nc.vector.tensor_reduce(
                    m8[:], logits_sb[:], axis=Ax.X, op=Alu.max)
                mall = pa_tiny.tile([P, 1], fp32, tag="mall")
                nc.gpsimd.partition_all_reduce(
                    mall[:], m8[:], channels=P, reduce_op=bass_isa.ReduceOp.max)
                negm = pa_tiny.tile([P, 1], fp32, tag="negm")
                nc.vector.tensor_scalar_mul(negm[:], mall[:], -1.0)
                ej = pa_tiny.tile([P, S_TILES], fp32, tag="ej")
                esum = pa_tiny.tile([P, 1], fp32, tag="esum")
                nc.scalar.activation(
                    ej[:], logits_sb[:], Act.Exp, bias=negm[:], scale=1.0,
                    accum_out=esum[:])
                etot = pa_tiny.tile([P, 1], fp32, tag="etot")
                nc.gpsimd.partition_all_reduce(
                    etot[:], esum[:], channels=P, reduce_op=bass_isa.ReduceOp.add)
                lnz = pa_tiny.tile([1, 1], fp32, tag="lnz")
                nc.scalar.activation(lnz[:], etot[0:1, :], Act.Ln)
                z1 = pa_tiny.tile([1, 1], fp32, tag="z1")
                nc.vector.tensor_add(z1[:], lnz[:], mall[0:1, :])
                z2 = pa_tiny.tile([1, 1], fp32, tag="z2")
                nc.vector.tensor_mul(z2[:], z1[:], z1[:])
                nc.sync.dma_start(zsq_o[:], z2[:])

            # ---------------- Phase B: gather + FFN + combine ----------------
            with (
                tc.tile_pool(name="pb_stage", bufs=3) as pb_stage,
                tc.tile_pool(name="pb_tokT", bufs=1) as pb_tokT,
                tc.tile_pool(name="pb_w1", bufs=2) as pb_w1,
                tc.tile_pool(name="pb_w2", bufs=2) as pb_w2,
                tc.tile_pool(name="pb_h", bufs=2) as pb_h,
                tc.tile_pool(name="pb_out2", bufs=1) as pb_out2,
                tc.tile_pool(name="pb_psh", bufs=2, space="PSUM") as pb_psh,
                tc.tile_pool(name="pb_pso", bufs=2, space="PSUM") as pb_pso,
            ):
                tokT = pb_tokT.tile([P, D_CHUNKS, KC], bf16, tag="tokT")
                out2 = pb_out2.tile([P, TOK_TILES, D], fp32, tag="out2")

                # gather + transpose into [D, tok] bf16 layout
                for c in range(TOK_TILES):
                    gt = pb_stage.tile([P, 1, D], fp32, tag="stage")
                    nc.gpsimd.dma_gather(
                        gt[:], hid[:], idx_rep[:, 8 * c:8 * (c + 1)],
                        num_idxs=P, num_idxs_reg=P, elem_size=D,
                    )
                    for grp in range(D_CHUNKS // 4):
                        pst = pb_psh.tile([P, 1024], fp32, tag="psh")
                        for q in range(4):
                            dc = grp * 4 + q
                            nc.tensor.transpose(
                                pst[:, q * P:(q + 1) * P],
                                gt[:, 0, dc * P:(dc + 1) * P], ident[:])
                        dst = tokT[:, grp * 4:(grp + 1) * 4, c * P:(c + 1) * P]
                        src = pst[:, 0:512].rearrange("p (q t) -> p q t", q=4)
                        if grp % 2 == 0:
                            nc.scalar.copy(dst, src)
                        else:
                            nc.vector.tensor_copy(dst, src)

                # FFN supers
                for sf in range(N_SUPER):
                    w1t = pb_w1.tile([P, D_CHUNKS, FS], bf16, tag="w1t")
                    nc.sync.dma_start(
                        w1t[:],
                        w1b.rearrange("(c p) f -> p c f", p=P)[
                            :, :, sf * FS:(sf + 1) * FS],
                    )
                    w2t = pb_w2.tile([P, FC_PER_SUPER, D], bf16, tag="w2t")
                    nc.sync.dma_start(
                        w2t[:],
                        w2b.rearrange("(c p) d -> p c d", p=P)[
                            :, sf * FC_PER_SUPER:(sf + 1) * FC_PER_SUPER, :],
                    )

                    hs = pb_h.tile([P, FC_PER_SUPER, KC], bf16, tag="hs")
                    for fc in range(FC_PER_SUPER):
                        psh = pb_psh.tile([P, 1024], fp32, tag="psh")
                        for dc in range(D_CHUNKS):
                            lw = w1t[:, dc, fc * P:(fc + 1) * P]
                            nc.tensor.matmul(
                                psh[:, 0:512], lw, tokT[:, dc, 0:512],
                                start=(dc == 0), stop=(dc == D_CHUNKS - 1))
                            nc.tensor.matmul(
                                psh[:, 512:1024], lw, tokT[:, dc, 512:1024],
                                start=(dc == 0), stop=(dc == D_CHUNKS - 1))
                        # silu(y) = y * sigmoid(y): ACT sigmoid + DVE multiply
                        sgb = pb_h.tile([P, 1024], bf16, tag="sgb")
                        nc.scalar.activation(sgb[:], psh[:], Act.Sigmoid)
                        nc.vector.tensor_mul(hs[:, fc, :], psh[:], sgb[:])

                    for c in range(TOK_TILES):
                        for dh in range(2):
                            pso = pb_pso.tile([P, 1024], fp32, tag="pso")
                            for fc in range(FC_PER_SUPER):
                                lw = hs[:, fc, c * P:(c + 1) * P]
                                nc.tensor.matmul(
                                    pso[:, 0:512], lw,
                                    w2t[:, fc, dh * 1024:dh * 1024 + 512],
                                    start=(fc == 0), stop=(fc == FC_PER_SUPER - 1))
                                nc.tensor.matmul(
                                    pso[:, 512:1024], lw,
                                    w2t[:, fc, dh * 1024 + 512:dh * 1024 + 1024],
                                    start=(fc == 0), stop=(fc == FC_PER_SUPER - 1))
                            dst = out2[:, c, dh * 1024:(dh + 1) * 1024]
                            if sf == 0:
                                nc.vector.tensor_copy(dst, pso[:])
                            else:
                                nc.vector.tensor_add(dst, dst, pso[:])

                # final rows = (tok + out2) * w  (transpose tok back from tokT)
                for c in range(TOK_TILES):
                    rows = pb_stage.tile([P, 1, D], fp32, tag="stage")
                    for dh in range(2):
                        psr = pb_pso.tile([P, 1024], bf16, tag="pso")
                        for q in range(8):
                            dc = dh * 8 + q
                            nc.tensor.transpose(
                                psr[:, q * P:(q + 1) * P],
                                tokT[:, dc, c * P:(c + 1) * P], identb[:])
                        seg = rows[:, 0, dh * 1024:(dh + 1) * 1024]
                        nc.vector.tensor_add(
                            seg, psr[:], out2[:, c, dh * 1024:(dh + 1) * 1024])
                        nc.vector.tensor_scalar_mul(seg, seg, w_col[:, c:c + 1])
                    nc.sync.dma_start(
                        rows_o.rearrange("(c p) d -> p c d", p=P)[:, c, :],
                        rows[:, 0, :])

    nc.compile()
    return nc


def _get_program():
    if "nc" not in _COMPILED:
        _COMPILED["nc"] = build_program()
    return _COMPILED["nc"]


def kernel(hidden_states, w_router, w1, w2):
    from concourse.bass_utils import run_bass_kernel_spmd

    hidden_states = np.asarray(hidden_states, np.float32)
    w_router = np.asarray(w_router, np.float32)
    w1b = np.asarray(w1, np.float32).astype(ml_dtypes.bfloat16)
    w2b = np.asarray(w2, np.float32).astype(ml_dtypes.bfloat16)

    nc = _get_program()

    in_maps = []
    for c in range(8):
        b, role = c // 2, c % 2
        con = np.zeros((P, 4), np.float32)
        con[:, 0] = role * KC
        con[:, 1] = role * KC + KC
        in_maps.append({
            "hid": np.ascontiguousarray(hidden_states[b]),
            "wr": w_router,
            "w1b": w1b,
            "w2b": w2b,
            "cons": con,
        })

    res = run_bass_kernel_spmd(nc, in_maps, core_ids=list(range(8))).results

    out = hidden_states.copy()
    for c in range(8):
        b = c // 2
        idx = res[c]["myidx"]
        out[b, idx, :] += res[c]["rows"]

    sel = np.stack([res[2 * b]["selall"] for b in range(B)]).astype(np.int32)
    zsq = np.stack([res[2 * b]["zsq"][0] for b in range(B)])
    zloss = np.float32(np.mean(zsq, dtype=np.float32))
    return out, zloss, sel


# revision 25
# speedup vs baseline: 1.0108x; 1.0108x over previous
"""Trainium2 Bass kernel for an expert-choice MoR block (B=4, S=4096, D=2048, F=8192).

Sharding: 8 cores = 4 batch rows x 2 token-halves. Each core redundantly
computes the router + top-k selection for its batch row on-device (pairwise
rank counting on DVE+ACT, gpsimd sparse_gather compaction), gathers its half
of the selected tokens by rank-range (indirect DMA), runs the SiLU FFN in
bf16 on the PE, applies the router weights, and emits its 1024 processed
rows + the index lists + the row's z-loss term. The host only splits inputs
and scatters the per-core rows back into a copy of hidden_states.
"""

import numpy as np
import ml_dtypes

B, S, D, F = 4, 4096, 2048, 8192
K = S // 2          # 2048 selected tokens per batch row
KC = K // 2         # 1024 tokens per core
P = 128
S_TILES = S // P    # 32
TOK_TILES = KC // P # 8
D_CHUNKS = D // P   # 16
N_SUPER = 16        # F split into supers of 512
FS = F // N_SUPER   # 512
FC_PER_SUPER = FS // P  # 4

_COMPILED = {}


def build_program():
    import concourse.bass as bass
    import concourse.tile as tile
    import concourse.bass_isa as bass_isa
    from concourse import mybir, bacc

    fp32 = mybir.dt.float32
    bf16 = mybir.dt.bfloat16
    i16 = mybir.dt.int16
    i32 = mybir.dt.int32
    u32 = mybir.dt.uint32
    Alu = mybir.AluOpType
    Act = mybir.ActivationFunctionType
    Ax = mybir.AxisListType

    nc = bacc.Bacc("TRN2", target_bir_lowering=False, debug=False)

    hid = nc.dram_tensor("hid", [S, D], fp32, kind="ExternalInput").ap()
    wr = nc.dram_tensor("wr", [D], fp32, kind="ExternalInput").ap()
    w1b = nc.dram_tensor("w1b", [D, F], bf16, kind="ExternalInput").ap()
    w2b = nc.dram_tensor("w2b", [F, D], bf16, kind="ExternalInput").ap()
    cons = nc.dram_tensor("cons", [P, 4], fp32, kind="ExternalInput").ap()

    rows_o = nc.dram_tensor("rows", [KC, D], fp32, kind="ExternalOutput").ap()
    myidx_o = nc.dram_tensor("myidx", [KC], i32, kind="ExternalOutput").ap()
    selall_o = nc.dram_tensor("selall", [K], i32, kind="ExternalOutput").ap()
    zsq_o = nc.dram_tensor("zsq", [1], fp32, kind="ExternalOutput").ap()

    with tile.TileContext(nc) as tc:
        with tc.tile_pool(name="persist", bufs=1) as pp:
            ident = pp.tile([P, P], fp32, tag="ident")
            identb = pp.tile([P, P], bf16, tag="identb")
            ones_pp = pp.tile([P, P], fp32, tag="ones_pp")
            nc.gpsimd.memset(ones_pp[:], 1.0)
            # identity: keep 1.0 where (partition - col) == 0
            nc.gpsimd.affine_select(
                ident[:], ones_pp[:], pattern=[[-1, P]], compare_op=Alu.is_equal,
                fill=0.0, base=0, channel_multiplier=1,
            )
            nc.vector.tensor_copy(identb[:], ident[:])

            consb = pp.tile([P, 4], fp32, tag="consb")
            nc.sync.dma_start(consb[:], cons[:])

            idx_rep = pp.tile([P, KC // 16], i16, tag="idx_rep")      # [128, 64]
            w_col = pp.tile([P, TOK_TILES], fp32, tag="w_col")        # [128, 8]
            logits_sb = pp.tile([P, S_TILES], fp32, tag="logits_sb")  # [128, 32]

            # ---------------- Phase A: router, ranks, selection ----------------
            with (
                tc.tile_pool(name="pa_big", bufs=3) as pa_big,
                tc.tile_pool(name="pa_bc", bufs=1) as pa_bc,
                tc.tile_pool(name="pa_junk", bufs=2) as pa_junk,
                tc.tile_pool(name="pa_junk2", bufs=2) as pa_junk2,
                tc.tile_pool(name="pa_small", bufs=1) as pa_small,
                tc.tile_pool(name="pa_tiny", bufs=2) as pa_tiny,
                tc.tile_pool(name="pa_psum", bufs=1, space="PSUM") as pa_psum,
            ):
                wr_row = pa_small.tile([1, D], fp32, tag="wr_row")
                nc.sync.dma_start(wr_row[:], wr[:])
                wrb = pa_bc.tile([P, D], fp32, tag="wrb")
                nc.gpsimd.partition_broadcast(wrb[:], wr_row[:], channels=P)

                # router logits: two-stage segmented reduction (bounds the
                # sequential-sum rounding walk). Two row-tiles per op group
                # to amortize DVE instruction overhead. Processed in two
                # 2048-token halves so the pairwise ranking of the first half
                # overlaps the router work of the second.
                HT = S_TILES // 2  # 16 tiles per half
                SH = S // 2        # 2048 tokens per half
                wrb2 = wrb[:].rearrange("p (t d) -> p t d", t=1).broadcast_to([P, 2, D])
                neg_logits = pa_small.tile([P, S_TILES], fp32, tag="neg_logits")
                hid_v = hid.rearrange("(i q p) d -> p i q d", p=P, q=2)
                lbc_halves = []

                def router_half(h):
                    for i in range(h * HT, (h + 1) * HT, 2):
                        ht = pa_big.tile([P, 2, D], fp32, tag="ht")
                        nc.sync.dma_start(ht[:], hid_v[:, i // 2])
                        prod = pa_big.tile([P, 2, D], fp32, tag="prod")
                        # 1 of 4 product groups on gpsimd, rest on DVE
                        if (i // 2) % 4 == 3:
                            nc.gpsimd.tensor_mul(prod[:], ht[:], wrb2)
                        else:
                            nc.vector.tensor_mul(prod[:], ht[:], wrb2)
                        # segmented accumulate on ACT (1024-wide chains)
                        part8 = pa_tiny.tile([P, 2, 2], fp32, tag="part8")
                        for t in range(2):
                            for sg in range(2):
                                jr = pa_tiny.tile([P, 1024], bf16, tag="jr")
                                nc.scalar.activation(
                                    jr[:], prod[:, t, sg * 1024:(sg + 1) * 1024],
                                    Act.Copy, bias=0.0, scale=1.0,
                                    accum_out=part8[:, t, sg:sg + 1],
                                )
                        nc.vector.tensor_reduce(
                            logits_sb[:, i:i + 2], part8[:], axis=Ax.X,
                            op=Alu.add,
                        )
                    nc.vector.tensor_scalar_mul(
                        neg_logits[:, h * HT:(h + 1) * HT],
                        logits_sb[:, h * HT:(h + 1) * HT], -1.0)
                    # flatten this half's logits to a row and broadcast
                    lrow = pa_small.tile([1, SH], fp32, tag=f"lrow{h}")
                    for j in range(4):
                        ps_row = pa_psum.tile([1, 512], fp32, tag="ps_row")
                        for q in range(4):
                            i = h * HT + 4 * j + q
                            nc.tensor.matmul(
                                ps_row[:, q * P:(q + 1) * P],
                                logits_sb[:, i:i + 1], ident[:],
                                start=True, stop=True,
                            )
                        nc.scalar.copy(lrow[:, j * 512:(j + 1) * 512], ps_row[:])
                    lbc = pa_bc.tile([P, SH], fp32, tag=f"lbc{h}")
                    nc.gpsimd.partition_broadcast(lbc[:], lrow[:], channels=P)
                    lbc_halves.append(lbc)

                # per-(i-tile, j-half) partial counts; ACT blocks use the
                # Sign trick, DVE blocks count is_gt directly.
                cnt = [pa_small.tile([P, HT], fp32, tag=f"cnt{jh}{half}",
                                     name=f"cnt{jh}{half}")
                       for jh in range(2) for half in range(2)]  # [jh*2+ih]

                def rank_block_act(itile, jh, out_col):
                    ja = pa_junk2.tile([P, SH], bf16, tag="ja")
                    nc.scalar.activation(
                        ja[:], lbc_halves[jh], Act.Sign,
                        bias=neg_logits[:, itile:itile + 1], scale=1.0,
                        accum_out=out_col,
                    )

                def rank_block_cnt(eng, itile, jh, out_col):
                    jd = pa_junk.tile([P, SH], bf16, tag="jd")
                    eng.tensor_scalar(
                        jd[:], lbc_halves[jh], logits_sb[:, itile:itile + 1],
                        None, op0=Alu.is_gt, op1=Alu.add, accum_out=out_col,
                    )

                router_half(0)
                router_half(1)
                # blocks A/C (DVE is_gt, exact counts): all i vs j-half 0
                for i in range(HT):
                    rank_block_cnt(nc.vector, i, 0, cnt[0][:, i:i + 1])
                for i in range(HT):
                    rank_block_cnt(nc.vector, HT + i, 0, cnt[1][:, i:i + 1])
                # blocks B/D (ACT sign): all i vs j-half 1
                for i in range(HT):
                    rank_block_act(i, 1, cnt[2][:, i:i + 1])
                for i in range(HT):
                    rank_block_act(HT + i, 1, cnt[3][:, i:i + 1])

                # sign-sum -> gt-count conversion: cnt = (sgn + SH - z) / 2
                # z = 1 when the i-tile's half equals the j-half (self term)
                nc.vector.tensor_scalar(
                    cnt[2][:], cnt[2][:], float(SH), 0.5,
                    op0=Alu.add, op1=Alu.mult)
                nc.vector.tensor_scalar(
                    cnt[3][:], cnt[3][:], float(SH - 1), 0.5,
                    op0=Alu.add, op1=Alu.mult)

                ranks = pa_small.tile([P, S_TILES], fp32, tag="ranks")
                nc.vector.tensor_add(ranks[:, 0:HT], cnt[0][:], cnt[2][:])
                nc.vector.tensor_add(ranks[:, HT:S_TILES], cnt[1][:], cnt[3][:])

                # probs = sigmoid(logits) * 0.1
                probs = pa_small.tile([P, S_TILES], fp32, tag="probs")
                nc.scalar.activation(probs[:], logits_sb[:], Act.Sigmoid)
                nc.vector.tensor_scalar_mul(probs[:], probs[:], 0.1)

                # --- critical path to the gather: my_mask -> my_idx -> idx_rep
                W16 = S // 16  # 256
                iota1_i = pa_small.tile([16, W16], i32, tag="iota1_i")
                nc.gpsimd.iota(
                    iota1_i[:], pattern=[[16, W16]], base=1, channel_multiplier=1)
                iota1 = pa_small.tile([16, W16], fp32, tag="iota1")
                nc.vector.tensor_copy(iota1[:], iota1_i[:])

                ranks_w = pa_small.tile([16, S // 16], fp32, tag="ranks_w")
                rw_v = ranks_w[:].rearrange("p (f g) -> p f g", g=8)
                for g in range(8):
                    nc.sync.dma_start(
                        rw_v[:, :, g], ranks[16 * g:16 * (g + 1), :])

                ge = pa_small.tile([16, W16], fp32, tag="ge")
                nc.vector.tensor_scalar(
                    ge[:], ranks_w[:], consb[0:16, 0:1], None, op0=Alu.is_ge)
                lt = pa_small.tile([16, W16], fp32, tag="lt")
                nc.vector.tensor_scalar(
                    lt[:], ranks_w[:], consb[0:16, 1:2], None, op0=Alu.is_lt)
                my_mask = pa_small.tile([16, W16], fp32, tag="my_mask")
                nc.vector.tensor_mul(my_mask[:], ge[:], lt[:])

                mmy = pa_small.tile([16, W16], fp32, tag="mmy")
                nc.vector.tensor_mul(mmy[:], iota1[:], my_mask[:])
                nc.vector.tensor_scalar_add(mmy[:], mmy[:], -1.0)
                my_idx_f = pa_small.tile([16, KC // 16], fp32, tag="my_idx_f")
                nf2 = pa_small.tile([1, 1], u32, tag="nf2")
                nc.gpsimd.sparse_gather(my_idx_f[:], mmy[:], num_found=nf2[:])
                my_idx_s = pa_small.tile([16, KC // 16], i16, tag="my_idx_s")
                nc.gpsimd.tensor_copy(my_idx_s[:], my_idx_f[:])
                for g in range(8):
                    nc.sync.dma_start(
                        idx_rep[16 * g:16 * (g + 1), :], my_idx_s[:])

                # --- off the critical path: sel/weights outputs
                probs_w = pa_small.tile([16, S // 16], fp32, tag="probs_w")
                pw_v = probs_w[:].rearrange("p (f g) -> p f g", g=8)
                for g in range(8):
                    nc.sync.dma_start(
                        pw_v[:, :, g], probs[16 * g:16 * (g + 1), :])
                sel_mask = pa_small.tile([16, W16], fp32, tag="sel_mask")
                nc.vector.tensor_scalar(
                    sel_mask[:], ranks_w[:], float(K), None, op0=Alu.is_lt)
                msel = pa_small.tile([16, W16], fp32, tag="msel")
                nc.vector.tensor_mul(msel[:], iota1[:], sel_mask[:])
                nc.vector.tensor_scalar_add(msel[:], msel[:], -1.0)
                # mprob = mask ? probs : -1  == (probs + 1) * mask - 1
                # (probs+1 rounds at ~1.2e-7 abs; negligible for the gating)
                mprob = pa_small.tile([16, W16], fp32, tag="mprob")
                nc.vector.tensor_scalar_add(mprob[:], probs_w[:], 1.0)
                nc.vector.tensor_mul(mprob[:], mprob[:], my_mask[:])
                nc.vector.tensor_scalar_add(mprob[:], mprob[:], -1.0)

                sel_all_f = pa_small.tile([16, K // 16], fp32, tag="sel_all_f")
                nf1 = pa_small.tile([1, 1], u32, tag="nf1")
                nc.gpsimd.sparse_gather(sel_all_f[:], msel[:], num_found=nf1[:])
                my_w_f = pa_small.tile([16, KC // 16], fp32, tag="my_w_f")
                nf3 = pa_small.tile([1, 1], u32, tag="nf3")
                nc.gpsimd.sparse_gather(my_w_f[:], mprob[:], num_found=nf3[:])

                sel_all_i = pa_small.tile([16, K // 16], i32, tag="sel_all_i")
                nc.vector.tensor_copy(sel_all_i[:], sel_all_f[:])
                nc.sync.dma_start(
                    selall_o.rearrange("(f p) -> p f", p=16), sel_all_i[:])
                my_idx_i = pa_small.tile([16, KC // 16], i32, tag="my_idx_i")
                nc.vector.tensor_copy(my_idx_i[:], my_idx_f[:])
                nc.sync.dma_start(
                    myidx_o.rearrange("(f p) -> p f", p=16), my_idx_i[:])

                # per-token weights wrapped-16 -> wrapped-128 column layout
                mw_v = my_w_f[:].rearrange("p (f g) -> p f g", g=8)
                for g in range(8):
                    nc.sync.dma_start(
                        w_col[16 * g:16 * (g + 1), :], mw_v[:, :, g])

                # z-loss = logsumexp(logits)^2
                m8 = pa_tiny.tile([P, 1], fp32, tag="m8")
                nc.vector.tensor_reduce(
                    m8[:], logits_sb[:], axis=Ax.X, op=Alu.max)
                mall = pa_tiny.tile([P, 1], fp32, tag="mall")
                nc.gpsimd.partition_all_reduce(
                    mall[:], m8[:], channels=P, reduce_op=bass_isa.ReduceOp.max)
                negm = pa_tiny.tile([P, 1], fp32, tag="negm")
                nc.vector.tensor_scalar_mul(negm[:], mall[:], -1.0)
                ej = pa_tiny.tile([P, S_TILES], fp32, tag="ej")
                esum = pa_tiny.tile([P, 1], fp32, tag="esum")
                nc.scalar.activation(
                    ej[:], logits_sb[:], Act.Exp, bias=negm[:], scale=1.0,
                    accum_out=esum[:])
                etot = pa_tiny.tile([P, 1], fp32, tag="etot")
                nc.gpsimd.partition_all_reduce(
                    etot[:], esum[:], channels=P, reduce_op=bass_isa.ReduceOp.add)
                lnz = pa_tiny.tile([1, 1], fp32, tag="lnz")
                nc.scalar.activation(lnz[:], etot[0:1, :], Act.Ln)
                z1 = pa_tiny.tile([1, 1], fp32, tag="z1")
                nc.vector.tensor_add(z1[:], lnz[:], mall[0:1, :])
                z2 = pa_tiny.tile([1, 1], fp32, tag="z2")
                nc.vector.tensor_mul(z2[:], z1[:], z1[:])
                nc.sync.dma_start(zsq_o[:], z2[:])

            # ---------------- Phase B: gather + FFN + combine ----------------
            with (
                tc.tile_pool(name="pb_stage", bufs=3) as pb_stage,
                tc.tile_pool(name="pb_tokT", bufs=1) as pb_tokT,
                tc.tile_pool(name="pb_w1", bufs=2) as pb_w1,
                tc.tile_pool(name="pb_w2", bufs=2) as pb_w2,
                tc.tile_pool(name="pb_h", bufs=2) as pb_h,
                tc.tile_pool(name="pb_out2", bufs=1) as pb_out2,
                tc.tile_pool(name="pb_psh", bufs=2, space="PSUM") as pb_psh,
                tc.tile_pool(name="pb_pso", bufs=2, space="PSUM") as pb_pso,
            ):
                tokT = pb_tokT.tile([P, D_CHUNKS, KC], bf16, tag="tokT")
                out2 = pb_out2.tile([P, TOK_TILES, D], fp32, tag="out2")

                # gather + transpose into [D, tok] bf16 layout
                for c in range(TOK_TILES):
                    gt = pb_stage.tile([P, 1, D], fp32, tag="stage")
                    nc.gpsimd.dma_gather(
                        gt[:], hid[:], idx_rep[:, 8 * c:8 * (c + 1)],
                        num_idxs=P, num_idxs_reg=P, elem_size=D,
                    )
                    for grp in range(D_CHUNKS // 4):
                        pst = pb_psh.tile([P, 1024], fp32, tag="psh")
                        for q in range(4):
                            dc = grp * 4 + q
                            nc.tensor.transpose(
                                pst[:, q * P:(q + 1) * P],
                                gt[:, 0, dc * P:(dc + 1) * P], ident[:])
                        dst = tokT[:, grp * 4:(grp + 1) * 4, c * P:(c + 1) * P]
                        src = pst[:, 0:512].rearrange("p (q t) -> p q t", q=4)
                        if grp % 2 == 0:
                            nc.scalar.copy(dst, src)
                        else:
                            nc.vector.tensor_copy(dst, src)

                # FFN supers
                for sf in range(N_SUPER):
                    w1t = pb_w1.tile([P, D_CHUNKS, FS], bf16, tag="w1t")
                    nc.sync.dma_start(
                        w1t[:],
                        w1b.rearrange("(c p) f -> p c f", p=P)[
                            :, :, sf * FS:(sf + 1) * FS],
                    )
                    w2t = pb_w2.tile([P, FC_PER_SUPER, D], bf16, tag="w2t")
                    nc.sync.dma_start(
                        w2t[:],
                        w2b.rearrange("(c p) d -> p c d", p=P)[
                            :, sf * FC_PER_SUPER:(sf + 1) * FC_PER_SUPER, :],
                    )

                    hs = pb_h.tile([P, FC_PER_SUPER, KC], bf16, tag="hs")
                    for fc in range(FC_PER_SUPER):
                        psh = pb_psh.tile([P, 1024], fp32, tag="psh")
                        for dc in range(D_CHUNKS):
                            lw = w1t[:, dc, fc * P:(fc + 1) * P]
                            nc.tensor.matmul(
                                psh[:, 0:512], lw, tokT[:, dc, 0:512],
                                start=(dc == 0), stop=(dc == D_CHUNKS - 1))
                            nc.tensor.matmul(
                                psh[:, 512:1024], lw, tokT[:, dc, 512:1024],
                                start=(dc == 0), stop=(dc == D_CHUNKS - 1))
                        # silu(y) = y * sigmoid(y): ACT sigmoid + DVE multiply
                        sgb = pb_h.tile([P, 1024], bf16, tag="sgb")
                        nc.scalar.activation(sgb[:], psh[:], Act.Sigmoid)
                        nc.vector.tensor_mul(hs[:, fc, :], psh[:], sgb[:])

                    for c in range(TOK_TILES):
                        for dh in range(2):
                            pso = pb_pso.tile([P, 1024], fp32, tag="pso")
                            for fc in range(FC_PER_SUPER):
                                lw = hs[:, fc, c * P:(c + 1) * P]
                                nc.tensor.matmul(
                                    pso[:, 0:512], lw,
                                    w2t[:, fc, dh * 1024:dh * 1024 + 512],
                                    start=(fc == 0), stop=(fc == FC_PER_SUPER - 1))
                                nc.tensor.matmul(
                                    pso[:, 512:1024], lw,
                                    w2t[:, fc, dh * 1024 + 512:dh * 1024 + 1024],
                                    start=(fc == 0), stop=(fc == FC_PER_SUPER - 1))
                            dst = out2[:, c, dh * 1024:(dh + 1) * 1024]
                            if sf == 0:
                                nc.vector.tensor_copy(dst, pso[:])
                            else:
                                nc.vector.tensor_add(dst, dst, pso[:])

                # final rows = (tok + out2) * w  (transpose tok back from tokT)
                for c in range(TOK_TILES):
                    rows = pb_stage.tile([P, 1, D], fp32, tag="stage")
                    for dh in range(2):
                        psr = pb_pso.tile([P, 1024], bf16, tag="pso")
                        for q in range(8):
                            dc = dh * 8 + q
                            nc.tensor.transpose(
                                psr[:, q * P:(q + 1) * P],
                                tokT[:, dc, c * P:(c + 1) * P], identb[:])
                        seg = rows[:, 0, dh * 1024:(dh + 1) * 1024]
                        nc.vector.tensor_add(
                            seg, psr[:], out2[:, c, dh * 1024:(dh + 1) * 1024])
                        nc.vector.tensor_scalar_mul(seg, seg, w_col[:, c:c + 1])
                    nc.sync.dma_start(
                        rows_o.rearrange("(c p) d -> p c d", p=P)[:, c, :],
                        rows[:, 0, :])

    nc.compile()
    return nc


def _get_program():
    if "nc" not in _COMPILED:
        _COMPILED["nc"] = build_program()
    return _COMPILED["nc"]


def kernel(hidden_states, w_router, w1, w2):
    from concourse.bass_utils import run_bass_kernel_spmd

    hidden_states = np.asarray(hidden_states, np.float32)
    w_router = np.asarray(w_router, np.float32)
    w1b = np.asarray(w1, np.float32).astype(ml_dtypes.bfloat16)
    w2b = np.asarray(w2, np.float32).astype(ml_dtypes.bfloat16)

    nc = _get_program()

    in_maps = []
    for c in range(8):
        b, role = c // 2, c % 2
        con = np.zeros((P, 4), np.float32)
        con[:, 0] = role * KC
        con[:, 1] = role * KC + KC
        in_maps.append({
            "hid": np.ascontiguousarray(hidden_states[b]),
            "wr": w_router,
            "w1b": w1b,
            "w2b": w2b,
            "cons": con,
        })

    res = run_bass_kernel_spmd(nc, in_maps, core_ids=list(range(8))).results

    out = hidden_states.copy()
    for c in range(8):
        b = c // 2
        idx = res[c]["myidx"]
        out[b, idx, :] += res[c]["rows"]

    sel = np.stack([res[2 * b]["selall"] for b in range(B)]).astype(np.int32)
    zsq = np.stack([res[2 * b]["zsq"][0] for b in range(B)])
    zloss = np.float32(np.mean(zsq, dtype=np.float32))
    return out, zloss, sel
